# revision 1
# baseline (speedup 1.0000x reference)
"""DiT block kernel for Trainium2, 8-core data-parallel over batch.

Layout strategy (per core, one batch element):
 - All activations FEATURE-major on chip: h^T [D(part), N(free)]; weights are
   host-pretransposed to W.T [in, out] bf16 so every linear is
   psum[o,n] = lhsT(W.T tile).T @ rhs(h^T)  with zero on-chip transposes.
 - V is produced TOKEN-major by swapping stationary/moving operands, giving
   the AV matmul's lhsT directly.  A ones-column appended to V computes the
   softmax denominator inside the same AV matmul.
 - Scores are built transposed (S.T [keys, queries]); softmax needs no max
   subtraction because rmsnorm + Cauchy-Schwarz bound |S| <= 8.
 - Partition-dim reductions (rmsnorm variance, softmax sums) via ones-vector
   matmuls; partition-dim broadcasts via rank-1 outer-product matmuls.

Host side (inside kernel(), off the HW clock): batch sharding, weight
transpose + bf16 cast, rope-table/mask preprocessing, output un-transpose.
"""
import sys
import numpy as np

sys.path.insert(0, "/opt/trn_rl_repo")

import ml_dtypes

import concourse.bass as bass
import concourse.tile as tile
import concourse.mybir as mybir

P = 128
N = 1024      # tokens
D = 1024      # model dim
H = 16        # heads
HD = 64       # head dim
KV = 128      # context tokens
MH = 4096     # mlp hidden
DT = D // P   # 8 d-tiles
NJ = N // 512 # 2 n-chunks
EPS = 1e-6

F32 = mybir.dt.float32
BF16 = mybir.dt.bfloat16
AF = mybir.ActivationFunctionType
ALU = mybir.AluOpType

_CACHE = {}


def split_multi_waits(nc, limit=1):
    """Walrus codegen accepts at most one sync wait per instruction; Tile's
    add_semaphores emits several.  Hoist extras onto same-engine NoOps placed
    immediately before the instruction (per-engine program order preserved)."""
    n_split = 0
    for f in nc.m.functions:
        for bb in f.blocks:
            insns = bb.instructions
            if not any(i.sync_info is not None and len(i.sync_info.on_wait) > limit
                       for i in insns):
                continue
            new = []
            for ins in insns:
                si = ins.sync_info
                if si is not None and len(si.on_wait) > limit:
                    waits = list(si.on_wait)
                    extra, keep = waits[:-limit], waits[-limit:]
                    for w in extra:
                        nop = mybir.InstNoOp(
                            name=nc.get_next_instruction_name(), ins=[], outs=[])
                        nop.engine = ins.engine
                        nop.sync_info = mybir.SyncInfo(on_wait=[w], on_update=[])
                        new.append(nop)
                    ins.sync_info = mybir.SyncInfo(
                        on_wait=keep, on_update=list(si.on_update))
                    n_split += 1
                new.append(ins)
            bb.instructions = new
    return n_split



def build_program(sim_safe=False, reps=1):
    nc = bass.Bass()

    def dram(name, shape, dt, out=False):
        return nc.declare_dram_parameter(name, list(shape), dt, isOutput=out)

    t = dict(
        xT=dram("xT", [D, N], F32),
        colpack=dram("colpack", [P, 81], F32),
        ctxT=dram("ctxT", [D, KV], BF16),
        cos2T=dram("cos2T", [P, N], F32),
        sin2T=dram("sin2T", [P, N], F32),
        adaT=dram("adaT", [D, 7 * D], BF16),
        rowf=dram("rowf", [1, 3 * D], F32),
        rowb=dram("rowb", [1, 5 * HD], BF16),
        qkvT=dram("qkvT", [D, 3 * D], BF16),
        projT=dram("projT", [D, D], BF16),
        cqT=dram("cqT", [D, D], BF16),
        ckT=dram("ckT", [D, D], BF16),
        cvT=dram("cvT", [D, D], BF16),
        cprojT=dram("cprojT", [D, D], BF16),
        w1T=dram("w1T", [D, MH], BF16),
        w3T=dram("w3T", [D, MH], BF16),
        w2T=dram("w2T", [MH, D], BF16),
        outT=dram("outT", [D, N], F32, out=True),
    )

    with tile.TileContext(nc) as tc:
        if reps > 1:
            with tc.For_i(0, reps):
                _emit(nc, tc, t, sim_safe)
        else:
            _emit(nc, tc, t, sim_safe)

    if not sim_safe:
        split_multi_waits(nc)
    return nc


def _emit(nc, tc, t, sim_safe=False):
    outT = t["outT"]
    dma = nc.sync.dma_start

    def silu(out, in_, tmp_pool, shape):
        if not sim_safe:
            nc.scalar.activation(out, in_, AF.Silu)
        else:
            sg = tmp_pool.tile(shape, F32, tag="sg_tmp", name="sg_tmp")
            nc.scalar.activation(sg, in_, AF.Sigmoid)
            nc.vector.tensor_mul(out, in_, sg)

    from contextlib import ExitStack
    es = ExitStack()
    pers = es.enter_context(tc.tile_pool(name="persist", bufs=1))
    vec = es.enter_context(tc.tile_pool(name="vec", bufs=1))
    hbuf = es.enter_context(tc.tile_pool(name="hbuf", bufs=1))

    # ---- residual stream (whole kernel) ----
    xt = []
    for j in range(DT):
        tl = pers.tile([P, N], F32, tag=f"xt{j}", name=f"xt{j}")
        dma(out=tl, in_=t["xT"][j * P:(j + 1) * P, :])
        xt.append(tl)
    h_bf = [hbuf.tile([P, N], BF16, tag=f"h{j}", name=f"h{j}") for j in range(DT)]

    colpack = vec.tile([P, 81], F32, tag="colpack", name="colpack")
    dma(out=colpack, in_=t["colpack"][:, :])
    cvt = [colpack[:, j:j + 1] for j in range(8)]
    projb_t = [colpack[:, 8 + j:9 + j] for j in range(DT)]
    cprojb_t = [colpack[:, 16 + j:17 + j] for j in range(DT)]
    adab_t = [colpack[:, 24 + j:25 + j] for j in range(56)]
    maskb_t = colpack[:, 80:81]

    rowf = vec.tile([1, 3 * D], F32, tag="rowf", name="rowf")
    dma(out=rowf, in_=t["rowf"][:, :])
    w_rows = {nm: rowf[:, i * D:(i + 1) * D]
              for i, nm in enumerate(("norm1_w", "normc_w", "norm2_w"))}
    rowb = vec.tile([1, 5 * HD], BF16, tag="rowb", name="rowb")
    dma(out=rowb, in_=t["rowb"][:, :])
    hd_rows = {nm: rowb[:, i * HD:(i + 1) * HD]
               for i, nm in enumerate(("qn_row", "kn_row", "cqn_row", "ckn_row"))}
    ones_row_bf = rowb[:, 4 * HD:5 * HD]

    ones_col_bf = vec.tile([P, 1], BF16, tag="ones_col", name="ones_col")
    nc.vector.memset(ones_col_bf, 1.0)
    eps_col = vec.tile([P, 1], F32, tag="eps_col", name="eps_col")
    nc.vector.memset(eps_col, EPS)

    # ---- adaLN: mod = adaT.T @ silu(c) + ada_b, 56 columns [P,1].
    # og 0-2 now (covers sh_msa/sc_msa for stage 1); og 3-7 deferred so
    # their adaT DMAs stream behind the qkv weight loads. ----
    mod = [None] * 56
    silu_c = [vec.tile([P, 1], BF16, tag=f"sc{j}", name=f"sc{j}")
              for j in range(DT)]
    for j in range(DT):
        silu(silu_c[j], cvt[j], vec, [P, 1])

    def ada_part(og_range, wbufs=2):
        # column i's 8 accumulating matmuls run to completion before column
        # i+1 starts: a psum bank allows only one open accumulation group.
        with tc.tile_pool(name="ada_w", bufs=wbufs) as awp, \
             tc.tile_pool(name="ada_ps", bufs=2, space="PSUM") as aps:
            for og in og_range:
                blks = []
                for kt in range(DT):
                    blk = awp.tile([P, 7 * P], BF16, tag=f"ablk{kt}", name=f"ablk{kt}")
                    dma(out=blk, in_=t["adaT"][kt * P:(kt + 1) * P,
                                               og * 7 * P:(og + 1) * 7 * P])
                    blks.append(blk)
                ps7 = aps.tile([P, 7], F32, tag="mps", name="mps")
                for i in range(7):
                    for kt in range(DT):
                        nc.tensor.matmul(ps7[:, i:i + 1], blks[kt][:, i * P:(i + 1) * P],
                                         silu_c[kt], start=(kt == 0), stop=(kt == DT - 1))
                for i in range(7):
                    ot = og * 7 + i
                    sb = vec.tile([P, 1], F32, tag=f"mod{ot}", name=f"mod{ot}")
                    nc.vector.tensor_add(sb, ps7[:, i:i + 1], adab_t[ot])
                    mod[ot] = sb

    ada_part(range(0, 3))
    sc1p = {"msa": [None] * DT, "mlp": [None] * DT}

    def sc1p_cols(nmq, q):
        for j in range(DT):
            tl = vec.tile([P, 1], F32, tag=f"sc1p_{nmq}{j}", name=f"sc1p_{nmq}{j}")
            nc.vector.tensor_scalar(out=tl, in0=mod[q * 8 + j], scalar1=1.0,
                                    scalar2=None, op0=ALU.add)
            sc1p[nmq][j] = tl

    sc1p_cols("msa", 1)
    pbg, cpbg = [None] * DT, [None] * DT

    def late_gate_cols():
        for j in range(DT):
            tl = vec.tile([P, 1], F32, tag=f"pbg{j}", name=f"pbg{j}")
            nc.vector.tensor_mul(tl, projb_t[j], mod[2 * 8 + j])
            pbg[j] = tl
            tl = vec.tile([P, 1], F32, tag=f"cpbg{j}", name=f"cpbg{j}")
            nc.vector.tensor_mul(tl, cprojb_t[j], mod[3 * 8 + j])
            cpbg[j] = tl
        sc1p_cols("mlp", 5)

    # ---- full-D rmsnorm into h_bf (optionally adaLN-modulated) ----
    def rmsnorm_full(w_name, mod_q=None, sh_cols=None):
        with tc.tile_pool(name="rn_tmp", bufs=2) as tmp, \
             tc.tile_pool(name="rn_ps", bufs=1, space="PSUM") as rps, \
             tc.tile_pool(name="rn_bc", bufs=2, space="PSUM") as bps:
            w_row = w_rows[w_name]
            ms = rps.tile([1, N], F32, tag="ms", name="ms")
            for j in range(DT):
                sq = tmp.tile([P, N], BF16, tag="sq", name="sq")
                nc.vector.tensor_mul(sq, xt[j], xt[j])
                for nj in range(NJ):
                    nsl = slice(nj * 512, (nj + 1) * 512)
                    nc.tensor.matmul(ms[:, nsl], ones_col_bf, sq[:, nsl],
                                     start=(j == 0), stop=(j == DT - 1))
            rstd = tmp.tile([1, N], F32, tag="rstd", name="rstd")
            nc.scalar.activation(rstd, ms, AF.Sqrt, bias=eps_col[0:1, :],
                                 scale=1.0 / D)
            nc.vector.reciprocal(rstd, rstd)
            for j in range(DT):
                bc = bps.tile([P, N], F32, tag="bc", name="bc")
                for nj in range(NJ):
                    nsl = slice(nj * 512, (nj + 1) * 512)
                    nc.tensor.matmul(bc[:, nsl], w_row[:, j * P:(j + 1) * P],
                                     rstd[:, nsl], start=True, stop=True)
                if mod_q is None:
                    nc.vector.tensor_mul(h_bf[j], xt[j], bc)
                else:
                    xn = tmp.tile([P, N], F32, tag="xn", name="xn")
                    nc.vector.tensor_mul(xn, xt[j], bc)
                    nc.vector.tensor_scalar(out=h_bf[j], in0=xn,
                                            scalar1=sc1p[mod_q][j],
                                            scalar2=sh_cols[j],
                                            op0=ALU.mult, op1=ALU.add)

    # ---- per-d-tile head pipeline: matmuls -> head rmsnorm [-> rope] ----
    def head_stage(emit_mm, w, jt, dst, row_bf, rope, pools, cos2=None, sin2=None):
        tmp, mmp, hsp, bqp = pools
        chunks = [(c0, min(512, w - c0)) for c0 in range(0, w, 512)]
        ps_list = []
        for nj, (c0, cw) in enumerate(chunks):
            ps = mmp.tile([P, 512], F32, tag="mm", name="mm")
            emit_mm(nj, ps[:, 0:cw])
            ps_list.append(ps)
        sq = tmp.tile([P, N], BF16, tag="sq", name="sq")
        for nj, (c0, cw) in enumerate(chunks):
            nc.scalar.activation(sq[:, c0:c0 + cw], ps_list[nj][:, 0:cw], AF.Square)
        hs = hsp.tile([HD + 1, N], F32, tag="hs", name="hs")
        for half in range(2):
            pf = slice(half * HD, (half + 1) * HD)
            po = half * HD
            for nj, (c0, cw) in enumerate(chunks):
                nc.tensor.matmul(hs[po:po + 1, c0:c0 + cw],
                                 ones_col_bf[pf, :], sq[pf, c0:c0 + cw],
                                 start=True, stop=True)
        bq = bqp.tile([P, N], F32, tag="bq", name="bq")
        for half in range(2):
            po = half * HD
            rt = tmp.tile([1, N], F32, tag="rqt", name="rqt")
            rb = tmp.tile([1, N], BF16, tag="rbt", name="rbt")
            nc.scalar.activation(rt[:, 0:w], hs[po:po + 1, 0:w], AF.Sqrt,
                                 bias=eps_col[0:1, :], scale=1.0 / HD)
            nc.vector.reciprocal(rt[:, 0:w], rt[:, 0:w])
            nc.vector.tensor_copy(rb[:, 0:w], rt[:, 0:w])
            pf = slice(half * HD, (half + 1) * HD)
            for nj, (c0, cw) in enumerate(chunks):
                nc.tensor.matmul(bq[pf, c0:c0 + cw], row_bf,
                                 rb[:, c0:c0 + cw], start=True, stop=True)
        if not rope:
            bq_sb = tmp.tile([P, N], F32, tag="bq_sb", name="bq_sb")
            for nj, (c0, cw) in enumerate(chunks):
                nc.scalar.copy(bq_sb[:, c0:c0 + cw], bq[:, c0:c0 + cw])
                nc.vector.tensor_mul(dst[:, c0:c0 + cw], ps_list[nj][:, 0:cw],
                                     bq_sb[:, c0:c0 + cw])
        else:
            # stage raw q/k into SBUF once (ACT) so the DVE rope chain runs
            # in 2x mode; split the rotate-half builds across ACT and DVE
            qsb = tmp.tile([P, N], F32, tag="qsb", name="qsb")
            rot = tmp.tile([P, N], F32, tag="rot", name="rot")
            m1 = tmp.tile([P, N], F32, tag="m1", name="m1")
            for nj, (c0, cw) in enumerate(chunks):
                nc.scalar.copy(qsb[:, c0:c0 + cw], ps_list[nj][:, 0:cw])
            for half in range(2):
                b = half * HD
                nc.vector.tensor_scalar(out=rot[b:b + 32, :], in0=qsb[b + 32:b + 64, :],
                                        scalar1=-1.0, scalar2=None, op0=ALU.mult)
                nc.scalar.copy(rot[b + 32:b + 64, :], qsb[b:b + 32, :])
            nc.vector.tensor_mul(m1, qsb, cos2)
            nc.vector.tensor_mul(rot, rot, sin2)
            nc.vector.tensor_add(m1, m1, rot)
            nc.vector.tensor_mul(dst, m1, bq)

    # ---- shared attention (self: n_mt=8, cross: n_mt=1) ----
    def attention(qsrc, ksrc, vtiles, n_mt, dst, mask_col, mid_cb=None):
        with tc.tile_pool(name="at_e", bufs=4) as ep, \
             tc.tile_pool(name="at_r", bufs=2) as rp, \
             tc.tile_pool(name="at_s", bufs=2, space="PSUM") as sps, \
             tc.tile_pool(name="at_o", bufs=2, space="PSUM") as ops, \
             tc.tile_pool(name="at_b", bufs=2, space="PSUM") as bps:
            for hh in range(H):
                if hh == 4 and mid_cb is not None:
                    mid_cb()
                jt, half = hh // 2, hh % 2
                hsl = slice(half * HD, (half + 1) * HD)
                for nj in range(NJ):
                    nsl = slice(nj * 512, (nj + 1) * 512)
                    o_ps = ops.tile([HD + 1, 512], F32, tag="ops", name="ops")
                    for mt in range(n_mt):
                        s_ps = sps.tile([P, 512], F32, tag="sps", name="sps")
                        lhs = ksrc[jt][hsl, mt * P:(mt + 1) * P]
                        nc.tensor.matmul(s_ps, lhs, qsrc[jt][hsl, nsl],
                                         start=True, stop=True)
                        e_bf = ep.tile([P, 512], BF16, tag="ebf", name="ebf")
                        if mask_col is None:
                            nc.scalar.activation(e_bf, s_ps, AF.Exp)
                        else:
                            nc.scalar.activation(e_bf, s_ps, AF.Exp, bias=mask_col)
                        nc.tensor.matmul(o_ps,
                                         vtiles[mt][:, hh * (HD + 1):(hh + 1) * (HD + 1)],
                                         e_bf, start=(mt == 0), stop=(mt == n_mt - 1))
                    r_f = rp.tile([1, 512], F32, tag="rf", name="rf")
                    nc.vector.reciprocal(r_f, o_ps[HD:HD + 1, :])
                    r_b = rp.tile([1, 512], BF16, tag="rb", name="rb")
                    nc.vector.tensor_copy(r_b, r_f)
                    br = bps.tile([HD, 512], F32, tag="br", name="br")
                    nc.tensor.matmul(br, ones_row_bf, r_b, start=True, stop=True)
                    br_sb = rp.tile([HD, 512], F32, tag="br_sb", name="br_sb")
                    nc.vector.tensor_copy(br_sb, br)
                    nc.vector.tensor_mul(dst[jt][hsl, nsl], o_ps[0:HD, :], br_sb)

    # ---- linear + gated residual into xt ----
    def linear_residual(wname, src_bf, gate_cols, bg_cols):
        with tc.tile_pool(name="wlin", bufs=1) as wp, \
             tc.tile_pool(name="lr_t", bufs=2) as tp, \
             tc.tile_pool(name="lr_ps", bufs=2, space="PSUM") as lps:
            w_sb = []
            for kt in range(DT):
                tl = wp.tile([P, D], BF16, tag=f"w{kt}", name=f"w{kt}")
                dma(out=tl, in_=t[wname][kt * P:(kt + 1) * P, :])
                w_sb.append(tl)
            for ot in range(DT):
                for nj in range(NJ):
                    nsl = slice(nj * 512, (nj + 1) * 512)
                    ps = lps.tile([P, 512], F32, tag="mm", name="mm")
                    for kt in range(DT):
                        nc.tensor.matmul(ps, w_sb[kt][:, ot * P:(ot + 1) * P],
                                         src_bf[kt][:, nsl],
                                         start=(kt == 0), stop=(kt == DT - 1))
                    tsb = tp.tile([P, 512], F32, tag="t", name="t")
                    nc.scalar.activation(tsb, ps, AF.Identity,
                                         bias=bg_cols[ot], scale=gate_cols[ot])
                    nc.vector.tensor_add(xt[ot][:, nsl], xt[ot][:, nsl], tsb)

    # =========== stage 1: norm1+mod, qkv, q/k norm+rope, V, self-attn ======
    rmsnorm_full("norm1_w", mod_q="msa", sh_cols=mod[0:8])

    with tc.tile_pool(name="s1", bufs=1) as s1:
        cos2 = s1.tile([P, N], F32, tag="cos2", name="cos2")
        sin2 = s1.tile([P, N], F32, tag="sin2", name="sin2")
        dma(out=cos2, in_=t["cos2T"][:, :])
        dma(out=sin2, in_=t["sin2T"][:, :])
        qhat = [s1.tile([P, N], BF16, tag=f"qh{j}", name=f"qh{j}") for j in range(DT)]
        khat = [s1.tile([P, N], BF16, tag=f"kh{j}", name=f"kh{j}") for j in range(DT)]
        vstore = [s1.tile([P, H * (HD + 1)], BF16, tag=f"v{j}", name=f"v{j}")
                  for j in range(DT)]

        with tc.tile_pool(name="wqkv", bufs=2) as wq:
            def load_section(sec):
                tiles = []
                for kt in range(DT):
                    tl = wq.tile([P, 1024], BF16, tag=f"qkvs{kt}", name=f"qkvs{kt}")
                    dma(out=tl, in_=t["qkvT"][kt * P:(kt + 1) * P,
                                              sec * 1024:(sec + 1) * 1024])
                    tiles.append(tl)
                return tiles

            with tc.tile_pool(name="qk_tmp", bufs=2) as tmp, \
                 tc.tile_pool(name="qk_mm", bufs=2, space="PSUM") as mmp, \
                 tc.tile_pool(name="qk_hs", bufs=2, space="PSUM") as hsp, \
                 tc.tile_pool(name="qk_bq", bufs=1, space="PSUM") as bqp:
                pools = (tmp, mmp, hsp, bqp)

                def v_block(mt, v_sb):
                    for vj in range(2):
                        ps = mmp.tile([P, 512], F32, tag="mm", name="mm")
                        for kt in range(DT):
                            nc.tensor.matmul(ps, h_bf[kt][:, mt * P:(mt + 1) * P],
                                             v_sb[kt][:, vj * 512:(vj + 1) * 512],
                                             start=(kt == 0), stop=(kt == DT - 1))
                        for i in range(8):
                            hh = vj * 8 + i
                            c0 = hh * (HD + 1)
                            nc.scalar.copy(vstore[mt][:, c0:c0 + HD],
                                           ps[:, i * HD:(i + 1) * HD])
                            nc.vector.memset(vstore[mt][:, c0 + HD:c0 + HD + 1], 1.0)

                qsec = load_section(0)
                vsec = load_section(2)
                for jt in range(DT):
                    def emm(nj, ps, jt=jt):
                        for kt in range(DT):
                            nc.tensor.matmul(ps, qsec[kt][:, jt * P:(jt + 1) * P],
                                             h_bf[kt][:, nj * 512:(nj + 1) * 512],
                                             start=(kt == 0), stop=(kt == DT - 1))
                    head_stage(emm, N, jt, qhat[jt], hd_rows["qn_row"],
                               True, pools, cos2, sin2)
                    v_block(jt, vsec)
                ksec = load_section(1)
                for jt in range(DT):
                    def emmk(nj, ps, jt=jt):
                        for kt in range(DT):
                            nc.tensor.matmul(ps, ksec[kt][:, jt * P:(jt + 1) * P],
                                             h_bf[kt][:, nj * 512:(nj + 1) * 512],
                                             start=(kt == 0), stop=(kt == DT - 1))
                    head_stage(emmk, N, jt, khat[jt], hd_rows["kn_row"],
                               True, pools, cos2, sin2)

        with tc.tile_pool(name="obuf1", bufs=1) as ob:
            o_bf = [ob.tile([P, N], BF16, tag=f"o{j}", name=f"o{j}") for j in range(DT)]
            def ada_rest():
                ada_part(range(3, 8), wbufs=2)
                late_gate_cols()
            attention(qhat, khat, vstore, DT, o_bf, None, mid_cb=ada_rest)
            linear_residual("projT", o_bf, mod[2 * 8:3 * 8], pbg)

    # =========== stage 2: cross-attention ===========
    rmsnorm_full("normc_w")   # h_bf <- hc

    with tc.tile_pool(name="s4", bufs=1) as s4:
        ctx_sb = [s4.tile([P, KV], BF16, tag=f"ctx{j}", name=f"ctx{j}")
                  for j in range(DT)]
        for j in range(DT):
            dma(out=ctx_sb[j], in_=t["ctxT"][j * P:(j + 1) * P, :])
        qc_hat = [s4.tile([P, N], BF16, tag=f"qch{j}", name=f"qch{j}")
                  for j in range(DT)]
        kc_hat = [s4.tile([P, KV], BF16, tag=f"kch{j}", name=f"kch{j}")
                  for j in range(DT)]
        vc_store = [s4.tile([KV, H * (HD + 1)], BF16, tag="vc", name="vc")]

        with tc.tile_pool(name="wc", bufs=1) as wc:
            c_sb = {}
            for nm in ("cqT", "ckT", "cvT"):
                rows = []
                for kt in range(DT):
                    tl = wc.tile([P, D], BF16, tag=f"{nm}{kt}", name=f"{nm}{kt}")
                    dma(out=tl, in_=t[nm][kt * P:(kt + 1) * P, :])
                    rows.append(tl)
                c_sb[nm] = rows

            with tc.tile_pool(name="cq_tmp", bufs=2) as tmp, \
                 tc.tile_pool(name="cq_mm", bufs=2, space="PSUM") as mmp, \
                 tc.tile_pool(name="cq_hs", bufs=2, space="PSUM") as hsp, \
                 tc.tile_pool(name="cq_bq", bufs=1, space="PSUM") as bqp:
                pools = (tmp, mmp, hsp, bqp)
                for jt in range(DT):
                    def emm(nj, ps, jt=jt):
                        for kt in range(DT):
                            nc.tensor.matmul(ps, c_sb["cqT"][kt][:, jt * P:(jt + 1) * P],
                                             h_bf[kt][:, nj * 512:(nj + 1) * 512],
                                             start=(kt == 0), stop=(kt == DT - 1))
                    head_stage(emm, N, jt, qc_hat[jt], hd_rows["cqn_row"],
                               False, pools)
                    def emmk(nj, ps, jt=jt):
                        for kt in range(DT):
                            nc.tensor.matmul(ps, c_sb["ckT"][kt][:, jt * P:(jt + 1) * P],
                                             ctx_sb[kt], start=(kt == 0), stop=(kt == DT - 1))
                    head_stage(emmk, KV, jt, kc_hat[jt], hd_rows["ckn_row"],
                               False, pools)
                for vj in range(2):
                    ps = mmp.tile([P, 512], F32, tag="mm", name="mm")
                    for kt in range(DT):
                        nc.tensor.matmul(ps, ctx_sb[kt],
                                         c_sb["cvT"][kt][:, vj * 512:(vj + 1) * 512],
                                         start=(kt == 0), stop=(kt == DT - 1))
                    for i in range(8):
                        hh = vj * 8 + i
                        c0 = hh * (HD + 1)
                        nc.scalar.copy(vc_store[0][:, c0:c0 + HD],
                                       ps[:, i * HD:(i + 1) * HD])
                        nc.vector.memset(vc_store[0][:, c0 + HD:c0 + HD + 1], 1.0)

        with tc.tile_pool(name="obuf2", bufs=1) as ob:
            o_bf = [ob.tile([P, N], BF16, tag=f"o{j}", name=f"o{j}") for j in range(DT)]
            attention(qc_hat, kc_hat, vc_store, 1, o_bf, maskb_t)
            linear_residual("cprojT", o_bf, mod[3 * 8:4 * 8], cpbg)

    # =========== stage 3: SwiGLU MLP ===========
    rmsnorm_full("norm2_w", mod_q="mlp", sh_cols=mod[4 * 8:5 * 8])

    with tc.tile_pool(name="gpool", bufs=1) as gp:
        g_tiles = [gp.tile([P, N], BF16, tag=f"g{mt}", name=f"g{mt}")
                   for mt in range(MH // P)]
        with tc.tile_pool(name="wmlp", bufs=2) as wp, \
             tc.tile_pool(name="ml_t", bufs=2) as tp, \
             tc.tile_pool(name="ml_ps", bufs=2, space="PSUM") as mps:
            for c in range(MH // 512):
                w1c, w3c = [], []
                for kt in range(DT):
                    tl = wp.tile([P, 512], BF16, tag=f"w1_{kt}", name=f"w1_{kt}")
                    dma(out=tl, in_=t["w1T"][kt * P:(kt + 1) * P, c * 512:(c + 1) * 512])
                    w1c.append(tl)
                    tl = wp.tile([P, 512], BF16, tag=f"w3_{kt}", name=f"w3_{kt}")
                    dma(out=tl, in_=t["w3T"][kt * P:(kt + 1) * P, c * 512:(c + 1) * 512])
                    w3c.append(tl)
                for i in range(4):
                    mt = c * 4 + i
                    isl = slice(i * P, (i + 1) * P)
                    for nj in range(NJ):
                        nsl = slice(nj * 512, (nj + 1) * 512)
                        ups = mps.tile([P, 512], F32, tag="ups", name="ups")
                        for kt in range(DT):
                            nc.tensor.matmul(ups, w1c[kt][:, isl], h_bf[kt][:, nsl],
                                             start=(kt == 0), stop=(kt == DT - 1))
                        su = tp.tile([P, 512], F32, tag="su", name="su")
                        silu(su, ups, tp, [P, 512])
                        tps = mps.tile([P, 512], F32, tag="tps", name="tps")
                        for kt in range(DT):
                            nc.tensor.matmul(tps, w3c[kt][:, isl], h_bf[kt][:, nsl],
                                             start=(kt == 0), stop=(kt == DT - 1))
                        nc.vector.tensor_mul(g_tiles[mt][:, nsl], su, tps)

        with tc.tile_pool(name="w2p", bufs=1) as w2p, \
             tc.tile_pool(name="fin_t", bufs=3) as tp, \
             tc.tile_pool(name="fin_ps", bufs=2, space="PSUM") as fps:
            w2_sb = []
            for mt in range(MH // P):
                tl = w2p.tile([P, D], BF16, tag=f"w2_{mt}", name=f"w2_{mt}")
                dma(out=tl, in_=t["w2T"][mt * P:(mt + 1) * P, :])
                w2_sb.append(tl)
            for ot in range(DT):
                for nj in range(NJ):
                    nsl = slice(nj * 512, (nj + 1) * 512)
                    ps = fps.tile([P, 512], F32, tag="yps", name="yps")
                    for mt in range(MH // P):
                        nc.tensor.matmul(ps, w2_sb[mt][:, ot * P:(ot + 1) * P],
                                         g_tiles[mt][:, nsl],
                                         start=(mt == 0), stop=(mt == MH // P - 1))
                    tsb = tp.tile([P, 512], F32, tag="t", name="t")
                    nc.scalar.activation(tsb, ps, AF.Identity, bias=0.0,
                                         scale=mod[6 * 8 + ot])
                    osb = tp.tile([P, 512], F32, tag="os", name="os")
                    nc.vector.tensor_add(osb, xt[ot][:, nsl], tsb)
                    dma(out=outT[ot * P:(ot + 1) * P, nsl], in_=osb)

    es.close()


# ===================== host side =====================

def _prep_inputs(inputs):
    bf = ml_dtypes.bfloat16
    f32 = np.float32

    def wT(a):
        return np.ascontiguousarray(np.asarray(a, f32).T).astype(bf)

    x = np.asarray(inputs["x"], f32)
    c = np.asarray(inputs["c"], f32)
    context = np.asarray(inputs["context"], f32)
    mask = np.asarray(inputs["context_mask"]).astype(bool)
    cos = np.asarray(inputs["rope_cos"], f32)   # [N, HD]
    sin = np.asarray(inputs["rope_sin"], f32)

    scale = HD ** -0.5
    rowf = np.concatenate([np.asarray(inputs[nm], f32).reshape(-1)
                           for nm in ("norm1_w", "normc_w", "norm2_w")]).reshape(1, -1)
    rowb = np.concatenate([
        np.asarray(inputs["qn_w"], f32).reshape(-1) * scale,
        np.asarray(inputs["kn_w"], f32).reshape(-1),
        np.asarray(inputs["cqn_w"], f32).reshape(-1) * scale,
        np.asarray(inputs["ckn_w"], f32).reshape(-1),
        np.ones(HD, f32),
    ]).reshape(1, -1).astype(bf)
    shared = {
        "cos2T": np.ascontiguousarray(np.concatenate([cos.T, cos.T], axis=0)),
        "sin2T": np.ascontiguousarray(np.concatenate([sin.T, sin.T], axis=0)),
        "adaT": wT(inputs["ada_w"]),
        "rowf": rowf,
        "rowb": rowb,
        "qkvT": wT(inputs["qkv_w"]),
        "projT": wT(inputs["proj_w"]),
        "cqT": wT(inputs["cq_w"]),
        "ckT": wT(inputs["ck_w"]),
        "cvT": wT(inputs["cv_w"]),
        "cprojT": wT(inputs["cproj_w"]),
        "w1T": wT(inputs["w1"]),
        "w3T": wT(inputs["w3"]),
        "w2T": wT(inputs["w2"]),
    }
    proj_b = np.asarray(inputs["proj_b"], f32).reshape(8, P).T       # [P, 8]
    cproj_b = np.asarray(inputs["cproj_b"], f32).reshape(8, P).T
    ada_b = np.asarray(inputs["ada_b"], f32).reshape(56, P).T        # [P, 56]

    in_maps = []
    for b in range(8):
        m = mask[b].copy()
        if not m.any():
            m[0] = True
        maskb = np.where(m, 0.0, -60.0).astype(f32).reshape(-1, 1)
        colpack = np.concatenate([
            c[b].reshape(8, P).T, proj_b, cproj_b, ada_b, maskb], axis=1)
        per = {
            "xT": np.ascontiguousarray(x[b].T),
            "colpack": np.ascontiguousarray(colpack.astype(f32)),
            "ctxT": np.ascontiguousarray(context[b].T).astype(bf),
        }
        per.update(shared)
        in_maps.append(per)
    return in_maps


def get_program():
    if "nc" not in _CACHE:
        _CACHE["nc"] = build_program()
    return _CACHE["nc"]


def kernel(**inputs):
    from concourse.bass_utils import run_bass_kernel_spmd
    nc = get_program()
    in_maps = _prep_inputs(inputs)
    res = run_bass_kernel_spmd(nc, in_maps, list(range(8)), trace=False)
    out = np.empty((8, N, D), np.float32)
    for b in range(8):
        out[b] = res.results[b]["outT"].T
    return out

